# revision 16
# baseline (speedup 1.0000x reference)
"""Bass/Tile kernel for nn_AttentionBlock (b,t,h,w,c = 1,8,64,64,512) on 8 TRN2 cores.

Sharding: 8 frames (b*t) -> one frame per NeuronCore, weights replicated,
no collectives (attention is independent per frame).

v2 notes (vs the baseline spill/flash kernel):
  - q stays RESIDENT in SBUF as fp16 channel-major (no DRAM spill/reload).
  - P and attention-output transposes run on the DMA xbar
    (dma_start_transpose, 16x128 tiles, bf16) instead of the PE array,
    removing ~1150 PE transposes and the PSUM->SBUF copies on VectorE.
  - softmax keeps the baseline flash scheme (logits have std ~512 in
    kernel units, so an exact full-row max bias is mandatory): wave A
    exps with bias -maxA overlap wave B's score matmuls; wave B exps
    use the full-row max; wave A is then rescaled by e^(maxA-max) and
    all 8 P slices are DMA-transposed afterwards.
  - xn is produced as f32r so the phase-1 PE transposes run at 1.5
    cycles/row instead of f32's 2.0.
  - proj/store for tile t runs two iterations later, hiding the DMA
    transpose latency of the attention output.
  - x tile 0 is DMA'd before the big weight loads so the norm pipeline
    starts immediately; wqk is loaded+converted in q/k halves so the q
    matmuls don't wait for the k weights.

float32r note: every tensor consumed by an f32r matmul must be *produced*
with dtype float32r by a compute engine (walrus rule), hence the
convert-copies for the weights and the f32r-producing normalize mul.
"""

import numpy as np
import ml_dtypes

B, T, H, W, C = 1, 8, 64, 64, 512
NTOK = H * W          # 4096 tokens per frame
P = 128
TT = NTOK // P        # 32 token tiles
NB = NTOK // 512      # 8 blocks of 512 tokens
CCH = C // P          # 4 channel chunks
EPS = 1e-6
N_CORES = 8

_COMPILED = None
LAST_EXEC_NS = None
TRACE = False


def _build():
    import concourse.bass as bass
    import concourse.tile as tile
    from concourse import mybir, bacc
    from concourse.masks import make_identity

    f32 = mybir.dt.float32
    f32r = mybir.dt.float32r
    bf16 = mybir.dt.bfloat16
    fp16 = mybir.dt.float16
    AF = mybir.ActivationFunctionType
    AX = mybir.AxisListType
    ALU = mybir.AluOpType

    nc = bacc.Bacc()
    x_d = nc.declare_dram_parameter("x", [NTOK, C], f32, isOutput=False)
    wqk_d = nc.declare_dram_parameter("w_qk", [C, 2 * C], f32, isOutput=False)
    wv_d = nc.declare_dram_parameter("w_v", [C, C], f32, isOutput=False)
    wp_d = nc.declare_dram_parameter("w_p", [C, C], bf16, isOutput=False)
    bqk_d = nc.declare_dram_parameter("b_qk", [P, 8], f32, isOutput=False)
    bv_d = nc.declare_dram_parameter("b_v", [P, C], f32, isOutput=False)
    bp_d = nc.declare_dram_parameter("b_p", [P, C], f32, isOutput=False)
    out_d = nc.declare_dram_parameter("out", [NTOK, C], f32, isOutput=True)

    from contextlib import ExitStack
    with tile.TileContext(nc) as tc:
        with ExitStack() as ctx:
            consts = ctx.enter_context(tc.tile_pool(name="consts", bufs=1))
            acts = ctx.enter_context(tc.tile_pool(name="acts", bufs=1))
            stagep = ctx.enter_context(tc.tile_pool(name="stage", bufs=1))
            bigp = ctx.enter_context(tc.tile_pool(name="big", bufs=2))
            ptp = ctx.enter_context(tc.tile_pool(name="ptr", bufs=2))
            xin = ctx.enter_context(tc.tile_pool(name="xin", bufs=2))
            facp = ctx.enter_context(tc.tile_pool(name="fac", bufs=4))
            xnp = ctx.enter_context(tc.tile_pool(name="xn", bufs=2))
            xnbp = ctx.enter_context(tc.tile_pool(name="xnb", bufs=2))
            smp = ctx.enter_context(tc.tile_pool(name="sm", bufs=3))
            rdp = ctx.enter_context(tc.tile_pool(name="rd", bufs=3))
            atp = ctx.enter_context(tc.tile_pool(name="at", bufs=2))
            atp2 = ctx.enter_context(tc.tile_pool(name="at2", bufs=2))
            xrp = ctx.enter_context(tc.tile_pool(name="xr", bufs=2))
            otp = ctx.enter_context(tc.tile_pool(name="ot", bufs=2))
            pss = ctx.enter_context(tc.tile_pool(name="ps_s", bufs=4, space="PSUM"))
            pst_p = ctx.enter_context(tc.tile_pool(name="ps_t", bufs=2, space="PSUM"))
            pso = ctx.enter_context(tc.tile_pool(name="ps_o", bufs=2, space="PSUM"))

            # ---------- first x tile + small consts before big weights ------
            xts = {}
            xts[0] = xin.tile([P, C], f32, tag="xt", name="xt0")
            nc.sync.dma_start(xts[0], x_d[0:P, :])
            bqk = consts.tile([P, 8], f32)
            nc.sync.dma_start(bqk, bqk_d[:, :])
            eps_t = consts.tile([P, 1], f32)
            nc.vector.memset(eps_t, EPS / C)
            ident_f = consts.tile([P, P], f32)
            make_identity(nc, ident_f)

            # wqk in f32r, staged through f32 in halves (q half first so the
            # q matmuls can start before the k weights even arrive).
            wqk = consts.tile([P, CCH, 2 * C], f32r)
            wst0 = stagep.tile([P, CCH, C], f32, tag="wst", name="wst0")
            nc.sync.dma_start(
                wst0, wqk_d[:, 0:C].rearrange("(cc p) d -> p cc d", p=P))
            nc.vector.tensor_copy(wqk[:, :, 0:C], wst0)

            for t in range(1, 4):
                xts[t] = xin.tile([P, C], f32, tag="xt", name=f"xt{t}")
                nc.sync.dma_start(xts[t], x_d[t * P:(t + 1) * P, :])

            wv = consts.tile([P, CCH, C], f32r)
            wvst = stagep.tile([P, CCH, C], f32, tag="wst", name="wstv")
            nc.sync.dma_start(wvst, wv_d.rearrange("(cc p) d -> p cc d", p=P))
            nc.vector.tensor_copy(wv, wvst)
            bv = consts.tile([P, C], f32)
            nc.sync.dma_start(bv, bv_d[:, :])

            wst1 = stagep.tile([P, CCH, C], f32, tag="wst", name="wst1")
            nc.sync.dma_start(
                wst1, wqk_d[:, C:2 * C].rearrange("(cc p) d -> p cc d", p=P))
            nc.vector.tensor_copy(wqk[:, :, C:2 * C], wst1)

            wp = consts.tile([P, CCH, C], bf16)
            nc.sync.dma_start(wp, wp_d.rearrange("(cc p) d -> p cc d", p=P))
            bp = consts.tile([P, C], f32)
            nc.sync.dma_start(bp, bp_d[:, :])

            # ---------- persistent activations ----------
            kT = acts.tile([P, CCH, NTOK], fp16)    # [c_part, c_chunk, tok]
            qT = acts.tile([P, CCH, NTOK], fp16)    # [c_part, c_chunk, tok]
            vv = acts.tile([P, TT, C], bf16)        # [tok_part, tok_tile, c]

            # ---------- phase 1: norm + QKV ----------
            for b in range(NB):
                xnb = xnbp.tile([P, CCH, 512], f32r)   # this block, channel-major
                for t4 in range(4):
                    t = b * 4 + t4
                    if t not in xts:
                        xts[t] = xin.tile([P, C], f32, tag="xt", name=f"xt{t}")
                        nc.sync.dma_start(xts[t], x_d[t * P:(t + 1) * P, :])
                    xt = xts.pop(t)
                    # prefetch next block's x tiles while this block computes
                    tn = t + 4
                    if tn < TT and tn not in xts:
                        xts[tn] = xin.tile([P, C], f32, tag="xt", name=f"xt{tn}")
                        nc.sync.dma_start(xts[tn], x_d[tn * P:(tn + 1) * P, :])
                    sq = pso.tile([P, C], f32, tag="o", name=f"sq{t}")
                    ssq = facp.tile([P, 1], f32, tag="ssq")
                    nc.scalar.activation(sq, xt, AF.Square, accum_out=ssq)
                    rmsn = facp.tile([P, 1], f32, tag="rmsn")
                    nc.scalar.activation(rmsn, ssq, AF.Sqrt,
                                         scale=1.0 / (C * C),
                                         bias=eps_t[:, 0:1])
                    fac = facp.tile([P, 1], f32, tag="fac")
                    nc.vector.reciprocal(fac, rmsn)    # sqrt(C)/rms
                    xnt = xnp.tile([P, C], f32)
                    nc.vector.tensor_scalar_mul(xnt, xt, fac)
                    ps = pst_p.tile([P, 512], f32, tag="t")
                    for cc in range(CCH):
                        nc.tensor.transpose(ps[:, cc * P:(cc + 1) * P],
                                            xnt[:, cc * P:(cc + 1) * P],
                                            ident_f)
                    nc.vector.tensor_copy(
                        xnb[:, :, t4 * P:(t4 + 1) * P],
                        ps.rearrange("p (cc j) -> p cc j", cc=CCH))
                # q and k, channel-major fp16, resident
                for m in range(CCH):
                    pq = pss.tile([P, 512], f32, tag="s", name=f"pq{m}")
                    for cc in range(CCH):
                        nc.tensor.matmul(pq, lhsT=wqk[:, cc, m * P:(m + 1) * P],
                                         rhs=xnb[:, cc, :],
                                         start=(cc == 0), stop=(cc == CCH - 1))
                    nc.scalar.activation(qT[:, m, b * 512:(b + 1) * 512], pq,
                                         AF.Identity, bias=bqk[:, m:m + 1])
                for m in range(CCH):
                    pk = pss.tile([P, 512], f32, tag="s", name=f"pk{m}")
                    for cc in range(CCH):
                        nc.tensor.matmul(
                            pk, lhsT=wqk[:, cc, C + m * P:C + (m + 1) * P],
                            rhs=xnb[:, cc, :],
                            start=(cc == 0), stop=(cc == CCH - 1))
                    nc.scalar.activation(kT[:, m, b * 512:(b + 1) * 512], pk,
                                         AF.Identity, bias=bqk[:, 4 + m:5 + m])
                for t4 in range(4):
                    t = b * 4 + t4
                    pv = pss.tile([P, 512], f32, tag="s", name=f"pv{t4}")
                    for cc in range(CCH):
                        nc.tensor.matmul(pv, lhsT=xnb[:, cc, t4 * P:(t4 + 1) * P],
                                         rhs=wv[:, cc, :],
                                         start=(cc == 0), stop=(cc == CCH - 1))
                    nc.vector.tensor_add(vv[:, t, :], pv, bv)

            # ---------- phase 2: attention + proj, pipelined per q-tile ----
            PTs = [None] * TT   # transposed P tiles [P, 8, CCH, P] bf16
            rds = [None] * TT   # 1/den per q-tile
            ats = [None] * TT   # transposed attention output [P, CCH, P] bf16
            pos = [None] * TT   # PV psum accumulators
            sm_state = {}

            def softmax_A(t):
                mx = smp.tile([P, 8], f32, tag="mx")
                dacc = smp.tile([P, 8], f32, tag="dacc")
                pb = bigp.tile([P, 8, 512], bf16, tag="pb")
                PTt = ptp.tile([P, 8, CCH, P], bf16, tag="pt")
                pscore = []
                for kb in range(4):
                    ps = pss.tile([P, 512], f32, tag="s", name=f"ps{kb}")
                    for cc in range(CCH):
                        nc.tensor.matmul(
                            ps, lhsT=qT[:, cc, t * P:(t + 1) * P],
                            rhs=kT[:, cc, kb * 512:(kb + 1) * 512],
                            start=(cc == 0), stop=(cc == CCH - 1))
                    nc.vector.tensor_reduce(mx[:, kb:kb + 1], ps,
                                            axis=AX.X, op=ALU.max)
                    pscore.append(ps)
                negA = smp.tile([P, 1], f32, tag="negA")
                nc.vector.tensor_reduce(negA, mx[:, 0:4], axis=AX.X,
                                        op=ALU.max, negate=True)
                for kb in range(4):
                    nc.scalar.activation(pb[:, kb, :], pscore[kb], AF.Exp,
                                         bias=negA,
                                         accum_out=dacc[:, kb:kb + 1])
                sm_state[t] = (pb, PTt, mx, negA, dacc)

            def softmax_B(t):
                pb, PTt, mx, negA, dacc = sm_state.pop(t)
                pscore = {}
                for kb in range(4, 8):
                    ps = pss.tile([P, 512], f32, tag="s", name=f"ps{kb}")
                    for cc in range(CCH):
                        nc.tensor.matmul(
                            ps, lhsT=qT[:, cc, t * P:(t + 1) * P],
                            rhs=kT[:, cc, kb * 512:(kb + 1) * 512],
                            start=(cc == 0), stop=(cc == CCH - 1))
                    nc.vector.tensor_reduce(mx[:, kb:kb + 1], ps,
                                            axis=AX.X, op=ALU.max)
                    pscore[kb] = ps
                # full-row max; wave B exps use it directly, wave A entries
                # are rescaled by e^(mA - m), then everything is transposed.
                negM = smp.tile([P, 1], f32, tag="negM")
                nc.vector.tensor_reduce(negM, mx, axis=AX.X,
                                        op=ALU.max, negate=True)
                for kb in range(4, 8):
                    nc.scalar.activation(pb[:, kb, :], pscore[kb], AF.Exp,
                                         bias=negM,
                                         accum_out=dacc[:, kb:kb + 1])
                    nc.sync.dma_start_transpose(PTt[:, kb], pb[:, kb, :])
                sdif = smp.tile([P, 1], f32, tag="sdif")
                nc.vector.tensor_tensor(sdif, negM, negA, ALU.subtract)
                scl = smp.tile([P, 1], f32, tag="scl")
                nc.scalar.activation(scl, sdif, AF.Exp)
                nc.vector.tensor_scalar_mul(pb[:, 0:4, :], pb[:, 0:4, :], scl)
                for kb in range(4):
                    nc.sync.dma_start_transpose(PTt[:, kb], pb[:, kb, :])
                dA = smp.tile([P, 1], f32, tag="dA")
                nc.vector.tensor_reduce(dA, dacc[:, 0:4], axis=AX.X,
                                        op=ALU.add)
                dB = smp.tile([P, 1], f32, tag="dB")
                nc.vector.tensor_reduce(dB, dacc[:, 4:8], axis=AX.X,
                                        op=ALU.add)
                den = smp.tile([P, 1], f32, tag="den")
                nc.vector.tensor_scalar(den, dA, scalar1=scl, scalar2=dB,
                                        op0=ALU.mult, op1=ALU.add)
                rd = rdp.tile([P, 1], f32)
                nc.vector.reciprocal(rd, den)
                rds[t] = rd
                PTs[t] = PTt

            def tail(t, half):
                PTt = PTs[t]
                po = pos[t]
                for kb in (range(4) if half == 0 else range(4, 8)):
                    for j in range(CCH):
                        jj = kb * 4 + j
                        nc.tensor.matmul(po, lhsT=PTt[:, kb, j, :],
                                         rhs=vv[:, jj, :],
                                         start=(jj == 0), stop=(jj == TT - 1))

            def fin_a(t):
                # normalize PV output and launch its transpose on the DMA xbar
                atok = atp.tile([P, C], bf16, tag="atok")
                nc.vector.tensor_scalar_mul(atok, pos[t], rds[t])
                at = atp2.tile([P, CCH, P], bf16, tag="at")
                nc.sync.dma_start_transpose(at, atok)
                ats[t] = at
                pos[t] = None
                rds[t] = None
                PTs[t] = None

            def fin_b(t):
                at = ats[t]
                pp = pss.tile([P, 512], f32, tag="s", name="pp")
                for m in range(CCH):
                    nc.tensor.matmul(pp, lhsT=at[:, m, :], rhs=wp[:, m, :],
                                     start=(m == 0), stop=(m == CCH - 1))
                xrt = xrp.tile([P, C], f32)
                nc.sync.dma_start(xrt, x_d[t * P:(t + 1) * P, :])
                ott = otp.tile([P, C], f32)
                nc.vector.tensor_add(ott, pp, bp)
                nc.vector.tensor_add(ott, ott, xrt)
                nc.sync.dma_start(out_d[t * P:(t + 1) * P, :], ott)
                ats[t] = None

            for t in range(TT + 2):
                if t < TT:
                    softmax_A(t)
                if 1 <= t <= TT:
                    pos[t - 1] = pso.tile([P, 512], f32, tag="o",
                                          name=f"po{t - 1}")
                    tail(t - 1, 0)
                if t >= 2:
                    fin_b(t - 2)
                if t < TT:
                    softmax_B(t)
                if 1 <= t <= TT:
                    tail(t - 1, 1)
                    fin_a(t - 1)
    nc.finalize()
    return nc


def _get_nc():
    global _COMPILED
    if _COMPILED is None:
        _COMPILED = _build()
    return _COMPILED


def kernel(x, scale, qkv_w, qkv_b, proj_w, proj_b):
    global LAST_EXEC_NS
    from concourse.bass_utils import run_bass_kernel_spmd

    x = np.asarray(x, dtype=np.float32)
    scale = np.asarray(scale, dtype=np.float32)
    qkv_w = np.asarray(qkv_w, dtype=np.float32)
    qkv_b = np.asarray(qkv_b, dtype=np.float32)
    proj_w = np.asarray(proj_w, dtype=np.float32)
    proj_b = np.asarray(proj_b, dtype=np.float32)

    # host prep: fold `scale` into qkv_w rows; fold attention 1/sqrt(c)
    # (c^-0.25 each) into Wq/Wk and their biases.
    s = C ** -0.25
    w_all = scale[:, None] * qkv_w            # [C, 3C]
    w_q = w_all[:, 0:C] * s
    w_k = w_all[:, C:2 * C] * s
    w_v = np.ascontiguousarray(w_all[:, 2 * C:3 * C], dtype=np.float32)
    b_q = qkv_b[0:C] * s
    b_k = qkv_b[C:2 * C] * s
    b_v = qkv_b[2 * C:3 * C]

    w_qk = np.ascontiguousarray(
        np.concatenate([w_q, w_k], axis=1), dtype=np.float32)
    w_p = proj_w.astype(ml_dtypes.bfloat16)
    b_qk = np.concatenate([b_q.reshape(4, P), b_k.reshape(4, P)], axis=0).T
    b_qk = np.ascontiguousarray(b_qk, dtype=np.float32)
    b_v_b = np.ascontiguousarray(np.broadcast_to(b_v, (P, C)), dtype=np.float32)
    b_p_b = np.ascontiguousarray(np.broadcast_to(proj_b, (P, C)),
                                 dtype=np.float32)

    frames = x.reshape(B * T, NTOK, C)
    in_maps = []
    for i in range(N_CORES):
        in_maps.append({
            "x": np.ascontiguousarray(frames[i]),
            "w_qk": w_qk, "w_v": w_v, "w_p": w_p,
            "b_qk": b_qk, "b_v": b_v_b, "b_p": b_p_b,
        })

    nc = _get_nc()
    res = run_bass_kernel_spmd(nc, in_maps, core_ids=list(range(N_CORES)),
                               trace=TRACE)
    LAST_EXEC_NS = res.exec_time_ns
    out = np.stack([np.asarray(res.results[i]["out"]) for i in range(N_CORES)])
    return out.reshape(B, T, H, W, C).astype(np.float32)


# revision 23
# speedup vs baseline: 1.1245x; 1.1245x over previous
"""Bass/Tile kernel for nn_AttentionBlock (b,t,h,w,c = 1,8,64,64,512) on 8 TRN2 cores.

Sharding: 8 frames (b*t) -> one frame per NeuronCore, weights replicated,
no collectives (attention is independent per frame).

v2 notes (vs the baseline spill/flash kernel):
  - q stays RESIDENT in SBUF as fp16 channel-major (no DRAM spill/reload).
  - P and attention-output transposes run on the DMA xbar
    (dma_start_transpose, 16x128 tiles, bf16) instead of the PE array,
    removing ~1150 PE transposes and the PSUM->SBUF copies on VectorE.
  - softmax keeps the baseline flash scheme (logits have std ~512 in
    kernel units, so an exact full-row max bias is mandatory): wave A
    exps with bias -maxA overlap wave B's score matmuls; wave B exps
    use the full-row max; wave A is then rescaled by e^(maxA-max) and
    all 8 P slices are DMA-transposed afterwards.
  - xn is produced as f32r so the phase-1 PE transposes run at 1.5
    cycles/row instead of f32's 2.0.
  - proj/store for tile t runs two iterations later, hiding the DMA
    transpose latency of the attention output.
  - x tile 0 is DMA'd before the big weight loads so the norm pipeline
    starts immediately; wqk is loaded+converted in q/k halves so the q
    matmuls don't wait for the k weights.

float32r note: every tensor consumed by an f32r matmul must be *produced*
with dtype float32r by a compute engine (walrus rule), hence the
convert-copies for the weights and the f32r-producing normalize mul.
"""

import numpy as np
import ml_dtypes

B, T, H, W, C = 1, 8, 64, 64, 512
NTOK = H * W          # 4096 tokens per frame
P = 128
TT = NTOK // P        # 32 token tiles
NB = NTOK // 512      # 8 blocks of 512 tokens
CCH = C // P          # 4 channel chunks
EPS = 1e-6
N_CORES = 8

_COMPILED = None
LAST_EXEC_NS = None
TRACE = False


def _build():
    import concourse.bass as bass
    import concourse.tile as tile
    from concourse import mybir, bacc
    from concourse.masks import make_identity

    f32 = mybir.dt.float32
    f32r = mybir.dt.float32r
    bf16 = mybir.dt.bfloat16
    fp16 = mybir.dt.float16
    AF = mybir.ActivationFunctionType
    AX = mybir.AxisListType
    ALU = mybir.AluOpType

    nc = bacc.Bacc()
    x_d = nc.declare_dram_parameter("x", [NTOK, C], f32, isOutput=False)
    wqk_d = nc.declare_dram_parameter("w_qk", [C, 2 * C], f32, isOutput=False)
    wv_d = nc.declare_dram_parameter("w_v", [C, C], f32, isOutput=False)
    wp_d = nc.declare_dram_parameter("w_p", [C, C], bf16, isOutput=False)
    bqk_d = nc.declare_dram_parameter("b_qk", [P, 8], f32, isOutput=False)
    bv_d = nc.declare_dram_parameter("b_v", [P, C], f32, isOutput=False)
    bp_d = nc.declare_dram_parameter("b_p", [P, C], f32, isOutput=False)
    out_d = nc.declare_dram_parameter("out", [NTOK, C], f32, isOutput=True)

    from contextlib import ExitStack
    with tile.TileContext(nc) as tc:
        with ExitStack() as ctx:
            consts = ctx.enter_context(tc.tile_pool(name="consts", bufs=1))
            acts = ctx.enter_context(tc.tile_pool(name="acts", bufs=1))
            stagep = ctx.enter_context(tc.tile_pool(name="stage", bufs=1))
            bigp = ctx.enter_context(tc.tile_pool(name="big", bufs=2))
            ptp = ctx.enter_context(tc.tile_pool(name="ptr", bufs=2))
            xin = ctx.enter_context(tc.tile_pool(name="xin", bufs=2))
            facp = ctx.enter_context(tc.tile_pool(name="fac", bufs=4))
            xnp = ctx.enter_context(tc.tile_pool(name="xn", bufs=2))
            xnbp = ctx.enter_context(tc.tile_pool(name="xnb", bufs=2))
            smp = ctx.enter_context(tc.tile_pool(name="sm", bufs=3))
            rdp = ctx.enter_context(tc.tile_pool(name="rd", bufs=3))
            atp = ctx.enter_context(tc.tile_pool(name="at", bufs=2))
            atmp = ctx.enter_context(tc.tile_pool(name="atm", bufs=1))
            atp2 = ctx.enter_context(tc.tile_pool(name="at2", bufs=2))
            xrp = ctx.enter_context(tc.tile_pool(name="xr", bufs=2))
            otp = ctx.enter_context(tc.tile_pool(name="ot", bufs=2))
            pss = ctx.enter_context(tc.tile_pool(name="ps_s", bufs=4, space="PSUM"))
            pst_p = ctx.enter_context(tc.tile_pool(name="ps_t", bufs=1, space="PSUM"))
            pso = ctx.enter_context(tc.tile_pool(name="ps_o", bufs=3, space="PSUM"))

            # ---------- first x tile + small consts before big weights ------
            xts = {}
            xts[0] = xin.tile([P, C], f32, tag="xt", name="xt0")
            nc.gpsimd.dma_start(xts[0], x_d[0:P, :])
            bqk = consts.tile([P, 8], f32)
            nc.sync.dma_start(bqk, bqk_d[:, :])
            eps_t = consts.tile([P, 1], f32)
            nc.vector.memset(eps_t, EPS / C)
            ident_f = consts.tile([P, P], f32)
            make_identity(nc, ident_f)

            # wqk in f32r, staged through f32 in halves (q half first so the
            # q matmuls can start before the k weights even arrive).
            wqk = consts.tile([P, CCH, 2 * C], f32r)
            wst0 = stagep.tile([P, CCH, C], f32, tag="wst", name="wst0")
            nc.sync.dma_start(
                wst0, wqk_d[:, 0:C].rearrange("(cc p) d -> p cc d", p=P))
            nc.vector.tensor_copy(wqk[:, :, 0:C], wst0)

            for t in range(1, 4):
                xts[t] = xin.tile([P, C], f32, tag="xt", name=f"xt{t}")
                nc.gpsimd.dma_start(xts[t], x_d[t * P:(t + 1) * P, :])

            wv = consts.tile([P, CCH, C], f32r)
            wvst = stagep.tile([P, CCH, C], f32, tag="wst", name="wstv")
            nc.sync.dma_start(wvst, wv_d.rearrange("(cc p) d -> p cc d", p=P))
            nc.vector.tensor_copy(wv, wvst)
            bv = consts.tile([P, C], f32)
            nc.sync.dma_start(bv, bv_d[:, :])

            wst1 = stagep.tile([P, CCH, C], f32, tag="wst", name="wst1")
            nc.sync.dma_start(
                wst1, wqk_d[:, C:2 * C].rearrange("(cc p) d -> p cc d", p=P))
            nc.vector.tensor_copy(wqk[:, :, C:2 * C], wst1)

            wp = consts.tile([P, CCH, C], bf16)
            nc.sync.dma_start(wp, wp_d.rearrange("(cc p) d -> p cc d", p=P))
            bp = consts.tile([P, C], f32)
            nc.sync.dma_start(bp, bp_d[:, :])

            # ---------- persistent activations ----------
            kT = acts.tile([P, CCH, NTOK], fp16)    # [c_part, c_chunk, tok]
            qT = acts.tile([P, CCH, NTOK], fp16)    # [c_part, c_chunk, tok]
            vv = acts.tile([P, TT, C], bf16)        # [tok_part, tok_tile, c]

            # ---------- phase 1: norm + QKV ----------
            for b in range(NB):
                xnb = xnbp.tile([P, CCH, 512], f32r)   # this block, channel-major
                for t4 in range(4):
                    t = b * 4 + t4
                    if t not in xts:
                        xts[t] = xin.tile([P, C], f32, tag="xt", name=f"xt{t}")
                        nc.gpsimd.dma_start(xts[t], x_d[t * P:(t + 1) * P, :])
                    xt = xts.pop(t)
                    # prefetch next block's x tiles while this block computes
                    tn = t + 4
                    if tn < TT and tn not in xts:
                        xts[tn] = xin.tile([P, C], f32, tag="xt", name=f"xt{tn}")
                        nc.gpsimd.dma_start(xts[tn], x_d[tn * P:(tn + 1) * P, :])
                    sq = pso.tile([P, C], f32, tag="o", name=f"sq{t}")
                    ssq = facp.tile([P, 1], f32, tag="ssq")
                    nc.scalar.activation(sq, xt, AF.Square, accum_out=ssq)
                    rmsn = facp.tile([P, 1], f32, tag="rmsn")
                    nc.scalar.activation(rmsn, ssq, AF.Sqrt,
                                         scale=1.0 / (C * C),
                                         bias=eps_t[:, 0:1])
                    fac = facp.tile([P, 1], f32, tag="fac")
                    nc.vector.reciprocal(fac, rmsn)    # sqrt(C)/rms
                    xnt = xnp.tile([P, C], f32)
                    nc.vector.tensor_scalar_mul(xnt, xt, fac)
                    ps = pst_p.tile([P, 512], f32, tag="t")
                    for cc in range(CCH):
                        nc.tensor.transpose(ps[:, cc * P:(cc + 1) * P],
                                            xnt[:, cc * P:(cc + 1) * P],
                                            ident_f)
                    nc.vector.tensor_copy(
                        xnb[:, :, t4 * P:(t4 + 1) * P],
                        ps.rearrange("p (cc j) -> p cc j", cc=CCH))
                # q and k, channel-major fp16, resident
                for m in range(CCH):
                    pq = pss.tile([P, 512], f32, tag="s", name=f"pq{m}")
                    for cc in range(CCH):
                        nc.tensor.matmul(pq, lhsT=wqk[:, cc, m * P:(m + 1) * P],
                                         rhs=xnb[:, cc, :],
                                         start=(cc == 0), stop=(cc == CCH - 1))
                    nc.scalar.activation(qT[:, m, b * 512:(b + 1) * 512], pq,
                                         AF.Identity, bias=bqk[:, m:m + 1])
                for m in range(CCH):
                    pk = pss.tile([P, 512], f32, tag="s", name=f"pk{m}")
                    for cc in range(CCH):
                        nc.tensor.matmul(
                            pk, lhsT=wqk[:, cc, C + m * P:C + (m + 1) * P],
                            rhs=xnb[:, cc, :],
                            start=(cc == 0), stop=(cc == CCH - 1))
                    nc.scalar.activation(kT[:, m, b * 512:(b + 1) * 512], pk,
                                         AF.Identity, bias=bqk[:, 4 + m:5 + m])
                for t4 in range(4):
                    t = b * 4 + t4
                    pv = pss.tile([P, 512], f32, tag="s", name=f"pv{t4}")
                    for cc in range(CCH):
                        nc.tensor.matmul(pv, lhsT=xnb[:, cc, t4 * P:(t4 + 1) * P],
                                         rhs=wv[:, cc, :],
                                         start=(cc == 0), stop=(cc == CCH - 1))
                    nc.vector.tensor_add(vv[:, t, :], pv, bv)

            # ---------- phase 2: attention + proj, pipelined per q-tile ----
            # Wave A P slices are written UNRESCALED (bias -maxA) and
            # transposed immediately; the flash correction e^(maxA-max) is
            # applied to the wave-A PV accumulator (po_A) at recombination
            # time, so no transpose ever waits on the rescale.
            PTs = [None] * TT   # transposed P tiles [P, 8, CCH, P] bf16
            rds = [None] * TT   # (scl*rd, rd) per q-tile
            ats = [None] * TT   # transposed attention output [P, CCH, P] bf16
            pos = [None] * TT   # (po_A, po_B) PV psum accumulators
            sm_state = {}

            def softmax_A(t):
                mx = smp.tile([P, 8], f32, tag="mx")
                dacc = smp.tile([P, 8], f32, tag="dacc")
                pb = bigp.tile([P, 8, 512], bf16, tag="pb")
                PTt = ptp.tile([P, 8, CCH, P], bf16, tag="pt")
                pscore = []
                for kb in range(4):
                    ps = pss.tile([P, 512], f32, tag="s", name=f"ps{kb}")
                    for cc in range(CCH):
                        nc.tensor.matmul(
                            ps, lhsT=qT[:, cc, t * P:(t + 1) * P],
                            rhs=kT[:, cc, kb * 512:(kb + 1) * 512],
                            start=(cc == 0), stop=(cc == CCH - 1))
                    nc.vector.tensor_reduce(mx[:, kb:kb + 1], ps,
                                            axis=AX.X, op=ALU.max)
                    pscore.append(ps)
                negA = smp.tile([P, 1], f32, tag="negA")
                nc.vector.tensor_reduce(negA, mx[:, 0:4], axis=AX.X,
                                        op=ALU.max, negate=True)
                for kb in range(4):
                    nc.scalar.activation(pb[:, kb, :], pscore[kb], AF.Exp,
                                         bias=negA,
                                         accum_out=dacc[:, kb:kb + 1])
                    nc.sync.dma_start_transpose(PTt[:, kb], pb[:, kb, :])
                sm_state[t] = (pb, PTt, mx, negA, dacc)

            def softmax_B(t):
                pb, PTt, mx, negA, dacc = sm_state.pop(t)
                pscore = {}
                for kb in range(4, 8):
                    ps = pss.tile([P, 512], f32, tag="s", name=f"ps{kb}")
                    for cc in range(CCH):
                        nc.tensor.matmul(
                            ps, lhsT=qT[:, cc, t * P:(t + 1) * P],
                            rhs=kT[:, cc, kb * 512:(kb + 1) * 512],
                            start=(cc == 0), stop=(cc == CCH - 1))
                    nc.vector.tensor_reduce(mx[:, kb:kb + 1], ps,
                                            axis=AX.X, op=ALU.max)
                    pscore[kb] = ps
                negM = smp.tile([P, 1], f32, tag="negM")
                nc.vector.tensor_reduce(negM, mx, axis=AX.X,
                                        op=ALU.max, negate=True)
                for kb in range(4, 8):
                    nc.scalar.activation(pb[:, kb, :], pscore[kb], AF.Exp,
                                         bias=negM,
                                         accum_out=dacc[:, kb:kb + 1])
                    nc.sync.dma_start_transpose(PTt[:, kb], pb[:, kb, :])
                sdif = smp.tile([P, 1], f32, tag="sdif")
                nc.vector.tensor_tensor(sdif, negM, negA, ALU.subtract)
                scl = smp.tile([P, 1], f32, tag="scl")
                nc.scalar.activation(scl, sdif, AF.Exp)
                dA = smp.tile([P, 1], f32, tag="dA")
                nc.vector.tensor_reduce(dA, dacc[:, 0:4], axis=AX.X,
                                        op=ALU.add)
                dB = smp.tile([P, 1], f32, tag="dB")
                nc.vector.tensor_reduce(dB, dacc[:, 4:8], axis=AX.X,
                                        op=ALU.add)
                den = smp.tile([P, 1], f32, tag="den")
                nc.vector.tensor_scalar(den, dA, scalar1=scl, scalar2=dB,
                                        op0=ALU.mult, op1=ALU.add)
                rd = rdp.tile([P, 1], f32, tag="rd")
                nc.vector.reciprocal(rd, den)
                srd = rdp.tile([P, 1], f32, tag="srd")
                nc.vector.tensor_tensor(srd, scl, rd, ALU.mult)
                rds[t] = (srd, rd)
                PTs[t] = PTt

            def tail(t, half):
                PTt = PTs[t]
                po = pos[t][half]
                for kb in (range(4) if half == 0 else range(4, 8)):
                    for j in range(CCH):
                        jj = kb * 4 + j
                        nc.tensor.matmul(po, lhsT=PTt[:, kb, j, :],
                                         rhs=vv[:, jj, :],
                                         start=(jj % 16 == 0),
                                         stop=(jj % 16 == 15))

            def fin_a(t):
                # attn = po_A * (scl/den) + po_B * (1/den), then transpose
                # channel-major on the DMA xbar for the projection matmul.
                srd, rd = rds[t]
                po_A, po_B = pos[t]
                tmpA = atmp.tile([P, C], bf16, tag="tmpA")
                nc.vector.tensor_scalar_mul(tmpA, po_A, srd)
                atok = atp.tile([P, C], bf16, tag="atok")
                nc.vector.tensor_scalar_mul(atok, po_B, rd)
                nc.vector.tensor_add(atok, atok, tmpA)
                at = atp2.tile([P, CCH, P], bf16, tag="at")
                nc.sync.dma_start_transpose(at, atok)
                ats[t] = at
                pos[t] = None
                rds[t] = None
                PTs[t] = None

            def fin_b(t):
                at = ats[t]
                pp = pss.tile([P, 512], f32, tag="s", name="pp")
                for m in range(CCH):
                    nc.tensor.matmul(pp, lhsT=at[:, m, :], rhs=wp[:, m, :],
                                     start=(m == 0), stop=(m == CCH - 1))
                xrt = xrp.tile([P, C], f32)
                nc.gpsimd.dma_start(xrt, x_d[t * P:(t + 1) * P, :])
                ott = otp.tile([P, C], f32)
                nc.vector.tensor_add(ott, pp, bp)
                nc.vector.tensor_add(ott, ott, xrt)
                nc.gpsimd.dma_start(out_d[t * P:(t + 1) * P, :], ott)
                ats[t] = None

            for t in range(TT + 2):
                if t < TT:
                    softmax_A(t)
                if 1 <= t <= TT:
                    pos[t - 1] = (
                        pso.tile([P, 512], f32, tag="o", name=f"poA{t - 1}"),
                        pso.tile([P, 512], f32, tag="o", name=f"poB{t - 1}"),
                    )
                    tail(t - 1, 0)
                if t >= 2:
                    fin_b(t - 2)
                if t < TT:
                    softmax_B(t)
                if 1 <= t <= TT:
                    tail(t - 1, 1)
                    fin_a(t - 1)
    nc.finalize()
    return nc


def _get_nc():
    global _COMPILED
    if _COMPILED is None:
        _COMPILED = _build()
    return _COMPILED


def kernel(x, scale, qkv_w, qkv_b, proj_w, proj_b):
    global LAST_EXEC_NS
    from concourse.bass_utils import run_bass_kernel_spmd

    x = np.asarray(x, dtype=np.float32)
    scale = np.asarray(scale, dtype=np.float32)
    qkv_w = np.asarray(qkv_w, dtype=np.float32)
    qkv_b = np.asarray(qkv_b, dtype=np.float32)
    proj_w = np.asarray(proj_w, dtype=np.float32)
    proj_b = np.asarray(proj_b, dtype=np.float32)

    # host prep: fold `scale` into qkv_w rows; fold attention 1/sqrt(c)
    # (c^-0.25 each) into Wq/Wk and their biases.
    s = C ** -0.25
    w_all = scale[:, None] * qkv_w            # [C, 3C]
    w_q = w_all[:, 0:C] * s
    w_k = w_all[:, C:2 * C] * s
    w_v = np.ascontiguousarray(w_all[:, 2 * C:3 * C], dtype=np.float32)
    b_q = qkv_b[0:C] * s
    b_k = qkv_b[C:2 * C] * s
    b_v = qkv_b[2 * C:3 * C]

    w_qk = np.ascontiguousarray(
        np.concatenate([w_q, w_k], axis=1), dtype=np.float32)
    w_p = proj_w.astype(ml_dtypes.bfloat16)
    b_qk = np.concatenate([b_q.reshape(4, P), b_k.reshape(4, P)], axis=0).T
    b_qk = np.ascontiguousarray(b_qk, dtype=np.float32)
    b_v_b = np.ascontiguousarray(np.broadcast_to(b_v, (P, C)), dtype=np.float32)
    b_p_b = np.ascontiguousarray(np.broadcast_to(proj_b, (P, C)),
                                 dtype=np.float32)

    frames = x.reshape(B * T, NTOK, C)
    in_maps = []
    for i in range(N_CORES):
        in_maps.append({
            "x": np.ascontiguousarray(frames[i]),
            "w_qk": w_qk, "w_v": w_v, "w_p": w_p,
            "b_qk": b_qk, "b_v": b_v_b, "b_p": b_p_b,
        })

    nc = _get_nc()
    res = run_bass_kernel_spmd(nc, in_maps, core_ids=list(range(N_CORES)),
                               trace=TRACE)
    LAST_EXEC_NS = res.exec_time_ns
    out = np.stack([np.asarray(res.results[i]["out"]) for i in range(N_CORES)])
    return out.reshape(B, T, H, W, C).astype(np.float32)


# revision 25
# speedup vs baseline: 1.2594x; 1.1199x over previous
"""Bass/Tile kernel for nn_AttentionBlock (b,t,h,w,c = 1,8,64,64,512) on 8 TRN2 cores.

Sharding: 8 frames (b*t) -> one frame per NeuronCore, weights replicated,
no collectives (attention is independent per frame).

v2 notes (vs the baseline spill/flash kernel):
  - q stays RESIDENT in SBUF as fp16 channel-major (no DRAM spill/reload).
  - P and attention-output transposes run on the DMA xbar
    (dma_start_transpose, 16x128 tiles, bf16) instead of the PE array,
    removing ~1150 PE transposes and the PSUM->SBUF copies on VectorE.
  - softmax keeps the baseline flash scheme (logits have std ~512 in
    kernel units, so an exact full-row max bias is mandatory): wave A
    exps with bias -maxA overlap wave B's score matmuls; wave B exps
    use the full-row max; wave A is then rescaled by e^(maxA-max) and
    all 8 P slices are DMA-transposed afterwards.
  - xn is produced as f32r so the phase-1 PE transposes run at 1.5
    cycles/row instead of f32's 2.0.
  - proj/store for tile t runs two iterations later, hiding the DMA
    transpose latency of the attention output.
  - x tile 0 is DMA'd before the big weight loads so the norm pipeline
    starts immediately; wqk is loaded+converted in q/k halves so the q
    matmuls don't wait for the k weights.

float32r note: every tensor consumed by an f32r matmul must be *produced*
with dtype float32r by a compute engine (walrus rule), hence the
convert-copies for the weights and the f32r-producing normalize mul.
"""

import numpy as np
import ml_dtypes

B, T, H, W, C = 1, 8, 64, 64, 512
NTOK = H * W          # 4096 tokens per frame
P = 128
TT = NTOK // P        # 32 token tiles
NB = NTOK // 512      # 8 blocks of 512 tokens
CCH = C // P          # 4 channel chunks
EPS = 1e-6
N_CORES = 8

_COMPILED = None
LAST_EXEC_NS = None
TRACE = False


def _build():
    import concourse.bass as bass
    import concourse.tile as tile
    from concourse import mybir, bacc
    from concourse.masks import make_identity

    f32 = mybir.dt.float32
    f32r = mybir.dt.float32r
    bf16 = mybir.dt.bfloat16
    fp16 = mybir.dt.float16
    AF = mybir.ActivationFunctionType
    AX = mybir.AxisListType
    ALU = mybir.AluOpType

    nc = bacc.Bacc()
    x_d = nc.declare_dram_parameter("x", [NTOK, C], f32, isOutput=False)
    wqk_d = nc.declare_dram_parameter("w_qk", [C, 2 * C], f32, isOutput=False)
    wv_d = nc.declare_dram_parameter("w_v", [C, C], f32, isOutput=False)
    wp_d = nc.declare_dram_parameter("w_p", [C, C], bf16, isOutput=False)
    bqk_d = nc.declare_dram_parameter("b_qk", [P, 8], f32, isOutput=False)
    bv_d = nc.declare_dram_parameter("b_v", [P, C], f32, isOutput=False)
    bp_d = nc.declare_dram_parameter("b_p", [P, C], f32, isOutput=False)
    out_d = nc.declare_dram_parameter("out", [NTOK, C], f32, isOutput=True)

    from contextlib import ExitStack
    with tile.TileContext(nc) as tc:
        with ExitStack() as ctx:
            consts = ctx.enter_context(tc.tile_pool(name="consts", bufs=1))
            acts = ctx.enter_context(tc.tile_pool(name="acts", bufs=1))
            stagep = ctx.enter_context(tc.tile_pool(name="stage", bufs=1))
            bigp = ctx.enter_context(tc.tile_pool(name="big", bufs=2))
            ptp = ctx.enter_context(tc.tile_pool(name="ptr", bufs=2))
            xin = ctx.enter_context(tc.tile_pool(name="xin", bufs=2))
            facp = ctx.enter_context(tc.tile_pool(name="fac", bufs=4))
            xnp = ctx.enter_context(tc.tile_pool(name="xn", bufs=2))
            xnbp = ctx.enter_context(tc.tile_pool(name="xnb", bufs=2))
            smp = ctx.enter_context(tc.tile_pool(name="sm", bufs=3))
            rdp = ctx.enter_context(tc.tile_pool(name="rd", bufs=3))
            atp = ctx.enter_context(tc.tile_pool(name="at", bufs=2))
            atmp = ctx.enter_context(tc.tile_pool(name="atm", bufs=1))
            atp2 = ctx.enter_context(tc.tile_pool(name="at2", bufs=2))
            xrp = ctx.enter_context(tc.tile_pool(name="xr", bufs=2))
            otp = ctx.enter_context(tc.tile_pool(name="ot", bufs=2))
            pss = ctx.enter_context(tc.tile_pool(name="ps_s", bufs=4, space="PSUM"))
            pst_p = ctx.enter_context(tc.tile_pool(name="ps_t", bufs=1, space="PSUM"))
            pso = ctx.enter_context(tc.tile_pool(name="ps_o", bufs=3, space="PSUM"))

            # ---------- first x tile + small consts before big weights ------
            xts = {}
            xts[0] = xin.tile([P, C], f32, tag="xt", name="xt0")
            nc.gpsimd.dma_start(xts[0], x_d[0:P, :])
            bqk = consts.tile([P, 8], f32)
            nc.sync.dma_start(bqk, bqk_d[:, :])
            eps_t = consts.tile([P, 1], f32)
            nc.vector.memset(eps_t, EPS / C)
            ident_f = consts.tile([P, P], f32)
            make_identity(nc, ident_f)

            # wqk in f32r, staged through f32 in halves (q half first so the
            # q matmuls can start before the k weights even arrive).
            wqk = consts.tile([P, CCH, 2 * C], f32r)
            wst0 = stagep.tile([P, CCH, C], f32, tag="wst", name="wst0")
            nc.sync.dma_start(
                wst0, wqk_d[:, 0:C].rearrange("(cc p) d -> p cc d", p=P))
            nc.vector.tensor_copy(wqk[:, :, 0:C], wst0)

            for t in range(1, 4):
                xts[t] = xin.tile([P, C], f32, tag="xt", name=f"xt{t}")
                nc.gpsimd.dma_start(xts[t], x_d[t * P:(t + 1) * P, :])

            wv = consts.tile([P, CCH, C], f32r)
            wvst = stagep.tile([P, CCH, C], f32, tag="wst", name="wstv")
            nc.sync.dma_start(wvst, wv_d.rearrange("(cc p) d -> p cc d", p=P))
            nc.vector.tensor_copy(wv, wvst)
            bv = consts.tile([P, C], f32)
            nc.sync.dma_start(bv, bv_d[:, :])

            wst1 = stagep.tile([P, CCH, C], f32, tag="wst", name="wst1")
            nc.sync.dma_start(
                wst1, wqk_d[:, C:2 * C].rearrange("(cc p) d -> p cc d", p=P))
            nc.vector.tensor_copy(wqk[:, :, C:2 * C], wst1)

            wp = consts.tile([P, CCH, C], bf16)
            nc.sync.dma_start(wp, wp_d.rearrange("(cc p) d -> p cc d", p=P))
            bp = consts.tile([P, C], f32)
            nc.sync.dma_start(bp, bp_d[:, :])

            # ---------- persistent activations ----------
            kT = acts.tile([P, CCH, NTOK], fp16)    # [c_part, c_chunk, tok]
            qT = acts.tile([P, CCH, NTOK], fp16)    # [c_part, c_chunk, tok]
            vv = acts.tile([P, TT, C], bf16)        # [tok_part, tok_tile, c]

            # ---------- phase 1: norm + QKV ----------
            for b in range(NB):
                xnb = xnbp.tile([P, CCH, 512], f32r)   # this block, channel-major
                for t4 in range(4):
                    t = b * 4 + t4
                    if t not in xts:
                        xts[t] = xin.tile([P, C], f32, tag="xt", name=f"xt{t}")
                        nc.gpsimd.dma_start(xts[t], x_d[t * P:(t + 1) * P, :])
                    xt = xts.pop(t)
                    # prefetch next block's x tiles while this block computes
                    tn = t + 4
                    if tn < TT and tn not in xts:
                        xts[tn] = xin.tile([P, C], f32, tag="xt", name=f"xt{tn}")
                        nc.gpsimd.dma_start(xts[tn], x_d[tn * P:(tn + 1) * P, :])
                    sq = pso.tile([P, C], f32, tag="o", name=f"sq{t}")
                    ssq = facp.tile([P, 1], f32, tag="ssq")
                    nc.scalar.activation(sq, xt, AF.Square, accum_out=ssq)
                    rmsn = facp.tile([P, 1], f32, tag="rmsn")
                    nc.scalar.activation(rmsn, ssq, AF.Sqrt,
                                         scale=1.0 / (C * C),
                                         bias=eps_t[:, 0:1])
                    fac = facp.tile([P, 1], f32, tag="fac")
                    nc.vector.reciprocal(fac, rmsn)    # sqrt(C)/rms
                    xnt = xnp.tile([P, C], f32)
                    nc.vector.tensor_scalar_mul(xnt, xt, fac)
                    ps = pst_p.tile([P, 512], f32, tag="t")
                    for cc in range(CCH):
                        nc.tensor.transpose(ps[:, cc * P:(cc + 1) * P],
                                            xnt[:, cc * P:(cc + 1) * P],
                                            ident_f)
                    nc.vector.tensor_copy(
                        xnb[:, :, t4 * P:(t4 + 1) * P],
                        ps.rearrange("p (cc j) -> p cc j", cc=CCH))
                # q and k, channel-major fp16, resident
                for m in range(CCH):
                    pq = pss.tile([P, 512], f32, tag="s", name=f"pq{m}")
                    for cc in range(CCH):
                        nc.tensor.matmul(pq, lhsT=wqk[:, cc, m * P:(m + 1) * P],
                                         rhs=xnb[:, cc, :],
                                         start=(cc == 0), stop=(cc == CCH - 1))
                    nc.scalar.activation(qT[:, m, b * 512:(b + 1) * 512], pq,
                                         AF.Identity, bias=bqk[:, m:m + 1])
                for m in range(CCH):
                    pk = pss.tile([P, 512], f32, tag="s", name=f"pk{m}")
                    for cc in range(CCH):
                        nc.tensor.matmul(
                            pk, lhsT=wqk[:, cc, C + m * P:C + (m + 1) * P],
                            rhs=xnb[:, cc, :],
                            start=(cc == 0), stop=(cc == CCH - 1))
                    nc.scalar.activation(kT[:, m, b * 512:(b + 1) * 512], pk,
                                         AF.Identity, bias=bqk[:, 4 + m:5 + m])
                for t4 in range(4):
                    t = b * 4 + t4
                    pv = pss.tile([P, 512], f32, tag="s", name=f"pv{t4}")
                    for cc in range(CCH):
                        nc.tensor.matmul(pv, lhsT=xnb[:, cc, t4 * P:(t4 + 1) * P],
                                         rhs=wv[:, cc, :],
                                         start=(cc == 0), stop=(cc == CCH - 1))
                    nc.vector.tensor_add(vv[:, t, :], pv, bv)

            # ---------- phase 2: attention + proj, pipelined per q-tile ----
            # Wave A P slices are written UNRESCALED (bias -maxA) and
            # transposed immediately; the flash correction e^(maxA-max) is
            # applied to the wave-A PV accumulator (po_A) at recombination
            # time, so no transpose ever waits on the rescale.
            PTs = [None] * TT   # transposed P tiles [P, 8, CCH, P] bf16
            rds = [None] * TT   # (scl*rd, rd) per q-tile
            ats = [None] * TT   # transposed attention output [P, CCH, P] bf16
            pos = [None] * TT   # (po_A, po_B) PV psum accumulators
            sm_state = {}

            def softmax_A(t):
                mx = smp.tile([P, 8], f32, tag="mx")
                dacc = smp.tile([P, 8], f32, tag="dacc")
                pb = bigp.tile([P, 8, 512], bf16, tag="pb")
                PTt = ptp.tile([P, 8, CCH, P], bf16, tag="pt")
                pscore = []
                for kb in range(4):
                    ps = pss.tile([P, 512], f32, tag="s", name=f"ps{kb}")
                    for cc in range(CCH):
                        nc.tensor.matmul(
                            ps, lhsT=qT[:, cc, t * P:(t + 1) * P],
                            rhs=kT[:, cc, kb * 512:(kb + 1) * 512],
                            start=(cc == 0), stop=(cc == CCH - 1))
                    nc.vector.tensor_reduce(mx[:, kb:kb + 1], ps,
                                            axis=AX.X, op=ALU.max)
                    pscore.append(ps)
                negA = smp.tile([P, 1], f32, tag="negA")
                nc.vector.tensor_reduce(negA, mx[:, 0:4], axis=AX.X,
                                        op=ALU.max, negate=True)
                for kb in range(4):
                    nc.scalar.activation(pb[:, kb, :], pscore[kb], AF.Exp,
                                         bias=negA,
                                         accum_out=dacc[:, kb:kb + 1])
                nc.sync.dma_start_transpose(PTt[:, 0:4], pb[:, 0:4, :])
                sm_state[t] = (pb, PTt, mx, negA, dacc)

            def softmax_B(t):
                pb, PTt, mx, negA, dacc = sm_state.pop(t)
                pscore = {}
                for kb in range(4, 8):
                    ps = pss.tile([P, 512], f32, tag="s", name=f"ps{kb}")
                    for cc in range(CCH):
                        nc.tensor.matmul(
                            ps, lhsT=qT[:, cc, t * P:(t + 1) * P],
                            rhs=kT[:, cc, kb * 512:(kb + 1) * 512],
                            start=(cc == 0), stop=(cc == CCH - 1))
                    nc.vector.tensor_reduce(mx[:, kb:kb + 1], ps,
                                            axis=AX.X, op=ALU.max)
                    pscore[kb] = ps
                negM = smp.tile([P, 1], f32, tag="negM")
                nc.vector.tensor_reduce(negM, mx, axis=AX.X,
                                        op=ALU.max, negate=True)
                for kb in range(4, 8):
                    nc.scalar.activation(pb[:, kb, :], pscore[kb], AF.Exp,
                                         bias=negM,
                                         accum_out=dacc[:, kb:kb + 1])
                nc.sync.dma_start_transpose(PTt[:, 4:8], pb[:, 4:8, :])
                sdif = smp.tile([P, 1], f32, tag="sdif")
                nc.vector.tensor_tensor(sdif, negM, negA, ALU.subtract)
                scl = smp.tile([P, 1], f32, tag="scl")
                nc.scalar.activation(scl, sdif, AF.Exp)
                dA = smp.tile([P, 1], f32, tag="dA")
                nc.vector.tensor_reduce(dA, dacc[:, 0:4], axis=AX.X,
                                        op=ALU.add)
                dB = smp.tile([P, 1], f32, tag="dB")
                nc.vector.tensor_reduce(dB, dacc[:, 4:8], axis=AX.X,
                                        op=ALU.add)
                den = smp.tile([P, 1], f32, tag="den")
                nc.vector.tensor_scalar(den, dA, scalar1=scl, scalar2=dB,
                                        op0=ALU.mult, op1=ALU.add)
                rd = rdp.tile([P, 1], f32, tag="rd")
                nc.vector.reciprocal(rd, den)
                srd = rdp.tile([P, 1], f32, tag="srd")
                nc.vector.tensor_tensor(srd, scl, rd, ALU.mult)
                rds[t] = (srd, rd)
                PTs[t] = PTt

            def tail(t, half):
                PTt = PTs[t]
                po = pos[t][half]
                for kb in (range(4) if half == 0 else range(4, 8)):
                    for j in range(CCH):
                        jj = kb * 4 + j
                        nc.tensor.matmul(po, lhsT=PTt[:, kb, j, :],
                                         rhs=vv[:, jj, :],
                                         start=(jj % 16 == 0),
                                         stop=(jj % 16 == 15))

            def fin_a(t):
                # attn = po_A * (scl/den) + po_B * (1/den), then transpose
                # channel-major on the DMA xbar for the projection matmul.
                srd, rd = rds[t]
                po_A, po_B = pos[t]
                tmpA = atmp.tile([P, C], bf16, tag="tmpA")
                nc.vector.tensor_scalar_mul(tmpA, po_A, srd)
                atok = atp.tile([P, C], bf16, tag="atok")
                nc.vector.tensor_scalar_mul(atok, po_B, rd)
                nc.vector.tensor_add(atok, atok, tmpA)
                at = atp2.tile([P, CCH, P], bf16, tag="at")
                nc.sync.dma_start_transpose(at, atok)
                ats[t] = at
                pos[t] = None
                rds[t] = None
                PTs[t] = None

            def fin_b(t):
                at = ats[t]
                pp = pss.tile([P, 512], f32, tag="s", name="pp")
                for m in range(CCH):
                    nc.tensor.matmul(pp, lhsT=at[:, m, :], rhs=wp[:, m, :],
                                     start=(m == 0), stop=(m == CCH - 1))
                xrt = xrp.tile([P, C], f32)
                nc.gpsimd.dma_start(xrt, x_d[t * P:(t + 1) * P, :])
                ott = otp.tile([P, C], f32)
                nc.vector.tensor_add(ott, pp, bp)
                nc.vector.tensor_add(ott, ott, xrt)
                nc.gpsimd.dma_start(out_d[t * P:(t + 1) * P, :], ott)
                ats[t] = None

            for t in range(TT + 2):
                if t < TT:
                    softmax_A(t)
                if 1 <= t <= TT:
                    pos[t - 1] = (
                        pso.tile([P, 512], f32, tag="o", name=f"poA{t - 1}"),
                        pso.tile([P, 512], f32, tag="o", name=f"poB{t - 1}"),
                    )
                    tail(t - 1, 0)
                if t >= 2:
                    fin_b(t - 2)
                if t < TT:
                    softmax_B(t)
                if 1 <= t <= TT:
                    tail(t - 1, 1)
                    fin_a(t - 1)
    nc.finalize()
    return nc


def _get_nc():
    global _COMPILED
    if _COMPILED is None:
        _COMPILED = _build()
    return _COMPILED


def kernel(x, scale, qkv_w, qkv_b, proj_w, proj_b):
    global LAST_EXEC_NS
    from concourse.bass_utils import run_bass_kernel_spmd

    x = np.asarray(x, dtype=np.float32)
    scale = np.asarray(scale, dtype=np.float32)
    qkv_w = np.asarray(qkv_w, dtype=np.float32)
    qkv_b = np.asarray(qkv_b, dtype=np.float32)
    proj_w = np.asarray(proj_w, dtype=np.float32)
    proj_b = np.asarray(proj_b, dtype=np.float32)

    # host prep: fold `scale` into qkv_w rows; fold attention 1/sqrt(c)
    # (c^-0.25 each) into Wq/Wk and their biases.
    s = C ** -0.25
    w_all = scale[:, None] * qkv_w            # [C, 3C]
    w_q = w_all[:, 0:C] * s
    w_k = w_all[:, C:2 * C] * s
    w_v = np.ascontiguousarray(w_all[:, 2 * C:3 * C], dtype=np.float32)
    b_q = qkv_b[0:C] * s
    b_k = qkv_b[C:2 * C] * s
    b_v = qkv_b[2 * C:3 * C]

    w_qk = np.ascontiguousarray(
        np.concatenate([w_q, w_k], axis=1), dtype=np.float32)
    w_p = proj_w.astype(ml_dtypes.bfloat16)
    b_qk = np.concatenate([b_q.reshape(4, P), b_k.reshape(4, P)], axis=0).T
    b_qk = np.ascontiguousarray(b_qk, dtype=np.float32)
    b_v_b = np.ascontiguousarray(np.broadcast_to(b_v, (P, C)), dtype=np.float32)
    b_p_b = np.ascontiguousarray(np.broadcast_to(proj_b, (P, C)),
                                 dtype=np.float32)

    frames = x.reshape(B * T, NTOK, C)
    in_maps = []
    for i in range(N_CORES):
        in_maps.append({
            "x": np.ascontiguousarray(frames[i]),
            "w_qk": w_qk, "w_v": w_v, "w_p": w_p,
            "b_qk": b_qk, "b_v": b_v_b, "b_p": b_p_b,
        })

    nc = _get_nc()
    res = run_bass_kernel_spmd(nc, in_maps, core_ids=list(range(N_CORES)),
                               trace=TRACE)
    LAST_EXEC_NS = res.exec_time_ns
    out = np.stack([np.asarray(res.results[i]["out"]) for i in range(N_CORES)])
    return out.reshape(B, T, H, W, C).astype(np.float32)


# revision 28
# speedup vs baseline: 1.2776x; 1.0145x over previous
"""Bass/Tile kernel for nn_AttentionBlock (b,t,h,w,c = 1,8,64,64,512) on 8 TRN2 cores.

Sharding: 8 frames (b*t) -> one frame per NeuronCore, weights replicated,
no collectives (attention is independent per frame).

v2 notes (vs the baseline spill/flash kernel):
  - q stays RESIDENT in SBUF as fp16 channel-major (no DRAM spill/reload).
  - P and attention-output transposes run on the DMA xbar
    (dma_start_transpose, 16x128 tiles, bf16) instead of the PE array,
    removing ~1150 PE transposes and the PSUM->SBUF copies on VectorE.
  - softmax keeps the baseline flash scheme (logits have std ~512 in
    kernel units, so an exact full-row max bias is mandatory): wave A
    exps with bias -maxA overlap wave B's score matmuls; wave B exps
    use the full-row max; wave A is then rescaled by e^(maxA-max) and
    all 8 P slices are DMA-transposed afterwards.
  - xn is produced as f32r so the phase-1 PE transposes run at 1.5
    cycles/row instead of f32's 2.0.
  - proj/store for tile t runs two iterations later, hiding the DMA
    transpose latency of the attention output.
  - x tile 0 is DMA'd before the big weight loads so the norm pipeline
    starts immediately; wqk is loaded+converted in q/k halves so the q
    matmuls don't wait for the k weights.

float32r note: every tensor consumed by an f32r matmul must be *produced*
with dtype float32r by a compute engine (walrus rule), hence the
convert-copies for the weights and the f32r-producing normalize mul.
"""

import numpy as np
import ml_dtypes

B, T, H, W, C = 1, 8, 64, 64, 512
NTOK = H * W          # 4096 tokens per frame
P = 128
TT = NTOK // P        # 32 token tiles
NB = NTOK // 512      # 8 blocks of 512 tokens
CCH = C // P          # 4 channel chunks
EPS = 1e-6
N_CORES = 8

_COMPILED = None
LAST_EXEC_NS = None
TRACE = False


def _build():
    import concourse.bass as bass
    import concourse.tile as tile
    from concourse import mybir, bacc
    from concourse.masks import make_identity

    f32 = mybir.dt.float32
    f32r = mybir.dt.float32r
    bf16 = mybir.dt.bfloat16
    fp16 = mybir.dt.float16
    AF = mybir.ActivationFunctionType
    AX = mybir.AxisListType
    ALU = mybir.AluOpType

    nc = bacc.Bacc()
    x_d = nc.declare_dram_parameter("x", [NTOK, C], f32, isOutput=False)
    wqk_d = nc.declare_dram_parameter("w_qk", [C, 2 * C], f32, isOutput=False)
    wv_d = nc.declare_dram_parameter("w_v", [C, C], f32, isOutput=False)
    wp_d = nc.declare_dram_parameter("w_p", [C, C], bf16, isOutput=False)
    bqk_d = nc.declare_dram_parameter("b_qk", [P, 8], f32, isOutput=False)
    bv_d = nc.declare_dram_parameter("b_v", [P, C], f32, isOutput=False)
    bp_d = nc.declare_dram_parameter("b_p", [P, C], f32, isOutput=False)
    out_d = nc.declare_dram_parameter("out", [NTOK, C], f32, isOutput=True)

    from contextlib import ExitStack
    with tile.TileContext(nc) as tc:
        with ExitStack() as ctx:
            consts = ctx.enter_context(tc.tile_pool(name="consts", bufs=1))
            acts = ctx.enter_context(tc.tile_pool(name="acts", bufs=1))
            stagep = ctx.enter_context(tc.tile_pool(name="stage", bufs=1))
            bigp = ctx.enter_context(tc.tile_pool(name="big", bufs=2))
            ptp = ctx.enter_context(tc.tile_pool(name="ptr", bufs=2))
            xin = ctx.enter_context(tc.tile_pool(name="xin", bufs=2))
            facp = ctx.enter_context(tc.tile_pool(name="fac", bufs=4))
            xnp = ctx.enter_context(tc.tile_pool(name="xn", bufs=2))
            xnbp = ctx.enter_context(tc.tile_pool(name="xnb", bufs=2))
            smp = ctx.enter_context(tc.tile_pool(name="sm", bufs=3))
            rdp = ctx.enter_context(tc.tile_pool(name="rd", bufs=3))
            atp = ctx.enter_context(tc.tile_pool(name="at", bufs=2))
            atmp = ctx.enter_context(tc.tile_pool(name="atm", bufs=1))
            atp2 = ctx.enter_context(tc.tile_pool(name="at2", bufs=2))
            xrp = ctx.enter_context(tc.tile_pool(name="xr", bufs=2))
            otp = ctx.enter_context(tc.tile_pool(name="ot", bufs=2))
            pss = ctx.enter_context(tc.tile_pool(name="ps_s", bufs=4, space="PSUM"))
            pso = ctx.enter_context(tc.tile_pool(name="ps_o", bufs=4, space="PSUM"))

            # ---------- first x tile + small consts before big weights ------
            xts = {}
            xts[0] = xin.tile([P, C], f32, tag="xt", name="xt0")
            nc.sync.dma_start(xts[0], x_d[0:P, :])
            bqk = consts.tile([P, 8], f32)
            nc.sync.dma_start(bqk, bqk_d[:, :])
            eps_t = consts.tile([P, 1], f32)
            nc.vector.memset(eps_t, EPS / C)
            ident_f = consts.tile([P, P], f32)
            make_identity(nc, ident_f)

            # wqk in f32r, staged through f32 in halves (q half first so the
            # q matmuls can start before the k weights even arrive).
            wqk = consts.tile([P, CCH, 2 * C], f32r)
            wst0 = stagep.tile([P, CCH, C], f32, tag="wst", name="wst0")
            nc.sync.dma_start(
                wst0, wqk_d[:, 0:C].rearrange("(cc p) d -> p cc d", p=P))
            nc.vector.tensor_copy(wqk[:, :, 0:C], wst0)

            for t in range(1, 4):
                xts[t] = xin.tile([P, C], f32, tag="xt", name=f"xt{t}")
                nc.gpsimd.dma_start(xts[t], x_d[t * P:(t + 1) * P, :])

            wv = consts.tile([P, CCH, C], f32r)
            wvst = stagep.tile([P, CCH, C], f32, tag="wst", name="wstv")
            nc.sync.dma_start(wvst, wv_d.rearrange("(cc p) d -> p cc d", p=P))
            nc.vector.tensor_copy(wv, wvst)
            bv = consts.tile([P, C], f32)
            nc.sync.dma_start(bv, bv_d[:, :])

            wst1 = stagep.tile([P, CCH, C], f32, tag="wst", name="wst1")
            nc.sync.dma_start(
                wst1, wqk_d[:, C:2 * C].rearrange("(cc p) d -> p cc d", p=P))
            nc.vector.tensor_copy(wqk[:, :, C:2 * C], wst1)

            wp = consts.tile([P, CCH, C], bf16)
            nc.sync.dma_start(wp, wp_d.rearrange("(cc p) d -> p cc d", p=P))
            bp = consts.tile([P, C], f32)
            nc.sync.dma_start(bp, bp_d[:, :])

            # ---------- persistent activations ----------
            kT = acts.tile([P, CCH, NTOK], fp16)    # [c_part, c_chunk, tok]
            qT = acts.tile([P, CCH, NTOK], fp16)    # [c_part, c_chunk, tok]
            vv = acts.tile([P, TT, C], bf16)        # [tok_part, tok_tile, c]

            # ---------- phase 1: norm + QKV ----------
            for b in range(NB):
                xnb = xnbp.tile([P, CCH, 512], f32r)   # this block, channel-major
                for t4 in range(4):
                    t = b * 4 + t4
                    if t not in xts:
                        xts[t] = xin.tile([P, C], f32, tag="xt", name=f"xt{t}")
                        nc.gpsimd.dma_start(xts[t], x_d[t * P:(t + 1) * P, :])
                    xt = xts.pop(t)
                    # prefetch next block's x tiles while this block computes
                    tn = t + 4
                    if tn < TT and tn not in xts:
                        xts[tn] = xin.tile([P, C], f32, tag="xt", name=f"xt{tn}")
                        nc.gpsimd.dma_start(xts[tn], x_d[tn * P:(tn + 1) * P, :])
                    sq = pso.tile([P, C], f32, tag="o", name=f"sq{t}")
                    ssq = facp.tile([P, 1], f32, tag="ssq")
                    nc.scalar.activation(sq, xt, AF.Square, accum_out=ssq)
                    rmsn = facp.tile([P, 1], f32, tag="rmsn")
                    nc.scalar.activation(rmsn, ssq, AF.Sqrt,
                                         scale=1.0 / (C * C),
                                         bias=eps_t[:, 0:1])
                    fac = facp.tile([P, 1], f32, tag="fac")
                    nc.vector.reciprocal(fac, rmsn)    # sqrt(C)/rms
                    xnt = xnp.tile([P, C], f32)
                    nc.vector.tensor_scalar_mul(xnt, xt, fac)
                    ps = pss.tile([P, 512], f32, tag="s", name=f"tr{t}")
                    for cc in range(CCH):
                        nc.tensor.transpose(ps[:, cc * P:(cc + 1) * P],
                                            xnt[:, cc * P:(cc + 1) * P],
                                            ident_f)
                    nc.vector.tensor_copy(
                        xnb[:, :, t4 * P:(t4 + 1) * P],
                        ps.rearrange("p (cc j) -> p cc j", cc=CCH))
                # q and k, channel-major fp16, resident
                for m in range(CCH):
                    pq = pss.tile([P, 512], f32, tag="s", name=f"pq{m}")
                    for cc in range(CCH):
                        nc.tensor.matmul(pq, lhsT=wqk[:, cc, m * P:(m + 1) * P],
                                         rhs=xnb[:, cc, :],
                                         start=(cc == 0), stop=(cc == CCH - 1))
                    nc.scalar.activation(qT[:, m, b * 512:(b + 1) * 512], pq,
                                         AF.Identity, bias=bqk[:, m:m + 1])
                for m in range(CCH):
                    pk = pss.tile([P, 512], f32, tag="s", name=f"pk{m}")
                    for cc in range(CCH):
                        nc.tensor.matmul(
                            pk, lhsT=wqk[:, cc, C + m * P:C + (m + 1) * P],
                            rhs=xnb[:, cc, :],
                            start=(cc == 0), stop=(cc == CCH - 1))
                    nc.scalar.activation(kT[:, m, b * 512:(b + 1) * 512], pk,
                                         AF.Identity, bias=bqk[:, 4 + m:5 + m])
                for t4 in range(4):
                    t = b * 4 + t4
                    pv = pss.tile([P, 512], f32, tag="s", name=f"pv{t4}")
                    for cc in range(CCH):
                        nc.tensor.matmul(pv, lhsT=xnb[:, cc, t4 * P:(t4 + 1) * P],
                                         rhs=wv[:, cc, :],
                                         start=(cc == 0), stop=(cc == CCH - 1))
                    nc.vector.tensor_add(vv[:, t, :], pv, bv)

            # ---------- phase 2: attention + proj, pipelined per q-tile ----
            # Wave A P slices are written UNRESCALED (bias -maxA) and
            # transposed immediately; the flash correction e^(maxA-max) is
            # applied to the wave-A PV accumulator (po_A) at recombination
            # time, so no transpose ever waits on the rescale.
            PTs = [None] * TT   # transposed P tiles [P, 8, CCH, P] bf16
            rds = [None] * TT   # (scl*rd, rd) per q-tile
            ats = [None] * TT   # transposed attention output [P, CCH, P] bf16
            pos = [None] * TT   # (po_A, po_B) PV psum accumulators
            sm_state = {}

            def softmax_A(t):
                mx = smp.tile([P, 8], f32, tag="mx")
                dacc = smp.tile([P, 8], f32, tag="dacc")
                pb = bigp.tile([P, 8, 512], bf16, tag="pb")
                PTt = ptp.tile([P, 8, CCH, P], bf16, tag="pt")
                pscore = []
                for kb in range(4):
                    ps = pss.tile([P, 512], f32, tag="s", name=f"ps{kb}")
                    for cc in range(CCH):
                        nc.tensor.matmul(
                            ps, lhsT=qT[:, cc, t * P:(t + 1) * P],
                            rhs=kT[:, cc, kb * 512:(kb + 1) * 512],
                            start=(cc == 0), stop=(cc == CCH - 1))
                    nc.vector.tensor_reduce(mx[:, kb:kb + 1], ps,
                                            axis=AX.X, op=ALU.max)
                    pscore.append(ps)
                negA = smp.tile([P, 1], f32, tag="negA")
                nc.vector.tensor_reduce(negA, mx[:, 0:4], axis=AX.X,
                                        op=ALU.max, negate=True)
                for kb in range(4):
                    nc.scalar.activation(pb[:, kb, :], pscore[kb], AF.Exp,
                                         bias=negA,
                                         accum_out=dacc[:, kb:kb + 1])
                nc.sync.dma_start_transpose(PTt[:, 0:4], pb[:, 0:4, :])
                sm_state[t] = (pb, PTt, mx, negA, dacc)

            def softmax_B(t):
                pb, PTt, mx, negA, dacc = sm_state.pop(t)
                pscore = {}
                for kb in range(4, 8):
                    ps = pss.tile([P, 512], f32, tag="s", name=f"ps{kb}")
                    for cc in range(CCH):
                        nc.tensor.matmul(
                            ps, lhsT=qT[:, cc, t * P:(t + 1) * P],
                            rhs=kT[:, cc, kb * 512:(kb + 1) * 512],
                            start=(cc == 0), stop=(cc == CCH - 1))
                    nc.vector.tensor_reduce(mx[:, kb:kb + 1], ps,
                                            axis=AX.X, op=ALU.max)
                    pscore[kb] = ps
                negM = smp.tile([P, 1], f32, tag="negM")
                nc.vector.tensor_reduce(negM, mx, axis=AX.X,
                                        op=ALU.max, negate=True)
                for kb in range(4, 8):
                    nc.scalar.activation(pb[:, kb, :], pscore[kb], AF.Exp,
                                         bias=negM,
                                         accum_out=dacc[:, kb:kb + 1])
                nc.sync.dma_start_transpose(PTt[:, 4:8], pb[:, 4:8, :])
                sdif = smp.tile([P, 1], f32, tag="sdif")
                nc.vector.tensor_tensor(sdif, negM, negA, ALU.subtract)
                scl = smp.tile([P, 1], f32, tag="scl")
                nc.scalar.activation(scl, sdif, AF.Exp)
                dA = smp.tile([P, 1], f32, tag="dA")
                nc.vector.tensor_reduce(dA, dacc[:, 0:4], axis=AX.X,
                                        op=ALU.add)
                dB = smp.tile([P, 1], f32, tag="dB")
                nc.vector.tensor_reduce(dB, dacc[:, 4:8], axis=AX.X,
                                        op=ALU.add)
                den = smp.tile([P, 1], f32, tag="den")
                nc.vector.tensor_scalar(den, dA, scalar1=scl, scalar2=dB,
                                        op0=ALU.mult, op1=ALU.add)
                rd = rdp.tile([P, 1], f32, tag="rd")
                nc.vector.reciprocal(rd, den)
                srd = rdp.tile([P, 1], f32, tag="srd")
                nc.vector.tensor_tensor(srd, scl, rd, ALU.mult)
                rds[t] = (srd, rd)
                PTs[t] = PTt

            def tail(t, half):
                PTt = PTs[t]
                po = pos[t][half]
                for kb in (range(4) if half == 0 else range(4, 8)):
                    for j in range(CCH):
                        jj = kb * 4 + j
                        nc.tensor.matmul(po, lhsT=PTt[:, kb, j, :],
                                         rhs=vv[:, jj, :],
                                         start=(jj % 16 == 0),
                                         stop=(jj % 16 == 15))

            def fin_a(t):
                # attn = po_A * (scl/den) + po_B * (1/den), then transpose
                # channel-major on the DMA xbar for the projection matmul.
                srd, rd = rds[t]
                po_A, po_B = pos[t]
                tmpA = atmp.tile([P, C], bf16, tag="tmpA")
                nc.vector.tensor_scalar_mul(tmpA, po_A, srd)
                atok = atp.tile([P, C], bf16, tag="atok")
                nc.vector.tensor_scalar_mul(atok, po_B, rd)
                nc.vector.tensor_add(atok, atok, tmpA)
                at = atp2.tile([P, CCH, P], bf16, tag="at")
                nc.sync.dma_start_transpose(at, atok)
                ats[t] = at
                pos[t] = None
                rds[t] = None
                PTs[t] = None

            def fin_b(t):
                at = ats[t]
                pp = pss.tile([P, 512], f32, tag="s", name="pp")
                for m in range(CCH):
                    nc.tensor.matmul(pp, lhsT=at[:, m, :], rhs=wp[:, m, :],
                                     start=(m == 0), stop=(m == CCH - 1))
                xrt = xrp.tile([P, C], f32)
                nc.gpsimd.dma_start(xrt, x_d[t * P:(t + 1) * P, :])
                ott = otp.tile([P, C], f32)
                nc.vector.tensor_add(ott, pp, bp)
                nc.vector.tensor_add(ott, ott, xrt)
                nc.gpsimd.dma_start(out_d[t * P:(t + 1) * P, :], ott)
                ats[t] = None

            for t in range(TT + 2):
                if t < TT:
                    softmax_A(t)
                if 1 <= t <= TT:
                    pos[t - 1] = (
                        pso.tile([P, 512], f32, tag="o", name=f"poA{t - 1}"),
                        pso.tile([P, 512], f32, tag="o", name=f"poB{t - 1}"),
                    )
                    tail(t - 1, 0)
                if t >= 2:
                    fin_b(t - 2)
                if t < TT:
                    softmax_B(t)
                if 1 <= t <= TT:
                    tail(t - 1, 1)
                    fin_a(t - 1)
    nc.finalize()
    return nc


def _get_nc():
    global _COMPILED
    if _COMPILED is None:
        _COMPILED = _build()
    return _COMPILED


def kernel(x, scale, qkv_w, qkv_b, proj_w, proj_b):
    global LAST_EXEC_NS
    from concourse.bass_utils import run_bass_kernel_spmd

    x = np.asarray(x, dtype=np.float32)
    scale = np.asarray(scale, dtype=np.float32)
    qkv_w = np.asarray(qkv_w, dtype=np.float32)
    qkv_b = np.asarray(qkv_b, dtype=np.float32)
    proj_w = np.asarray(proj_w, dtype=np.float32)
    proj_b = np.asarray(proj_b, dtype=np.float32)

    # host prep: fold `scale` into qkv_w rows; fold attention 1/sqrt(c)
    # (c^-0.25 each) into Wq/Wk and their biases.
    s = C ** -0.25
    w_all = scale[:, None] * qkv_w            # [C, 3C]
    w_q = w_all[:, 0:C] * s
    w_k = w_all[:, C:2 * C] * s
    w_v = np.ascontiguousarray(w_all[:, 2 * C:3 * C], dtype=np.float32)
    b_q = qkv_b[0:C] * s
    b_k = qkv_b[C:2 * C] * s
    b_v = qkv_b[2 * C:3 * C]

    w_qk = np.ascontiguousarray(
        np.concatenate([w_q, w_k], axis=1), dtype=np.float32)
    w_p = proj_w.astype(ml_dtypes.bfloat16)
    b_qk = np.concatenate([b_q.reshape(4, P), b_k.reshape(4, P)], axis=0).T
    b_qk = np.ascontiguousarray(b_qk, dtype=np.float32)
    b_v_b = np.ascontiguousarray(np.broadcast_to(b_v, (P, C)), dtype=np.float32)
    b_p_b = np.ascontiguousarray(np.broadcast_to(proj_b, (P, C)),
                                 dtype=np.float32)

    frames = x.reshape(B * T, NTOK, C)
    in_maps = []
    for i in range(N_CORES):
        in_maps.append({
            "x": np.ascontiguousarray(frames[i]),
            "w_qk": w_qk, "w_v": w_v, "w_p": w_p,
            "b_qk": b_qk, "b_v": b_v_b, "b_p": b_p_b,
        })

    nc = _get_nc()
    res = run_bass_kernel_spmd(nc, in_maps, core_ids=list(range(N_CORES)),
                               trace=TRACE)
    LAST_EXEC_NS = res.exec_time_ns
    out = np.stack([np.asarray(res.results[i]["out"]) for i in range(N_CORES)])
    return out.reshape(B, T, H, W, C).astype(np.float32)
